# revision 5
# baseline (speedup 1.0000x reference)
"""Causal self-attention (B=4, T=2048, C=1024, H=16) on 8 trn2 NeuronCores.

Sharding: core c -> batch b = c//2, heads h0 = (c%2)*8 .. h0+8 (tensor
parallel over heads: c_attn columns / c_proj rows split). Each core computes a
partial projection output [T, C] in bf16; the host sums the two partials per
batch and adds b_proj.

Device-side dataflow (fp8e4 DoubleRow for q/k projections and PV; bf16 for
v/S/proj):
  - host passes x[b] pre-transposed twice: xt [C, T] bf16 (v path) and
    xt8 [C, T] fp8e4 (q/k path); wq/wk are fp8e4 scaled by 32
  - q/k projections: DoubleRow fp8 matmuls, contraction 256 per instruction
    (kc-pair tiles [128, 2, *]); PSUM evacuated on DVE with tensor_scalar
    (scale 1/32 + bq bias for q; bk is DROPPED entirely - a per-query-constant
    logit shift cancels in softmax)
  - qT, kT [128, NP, T] bf16: head pair 2m/2m+1 stacked on partitions
    0:64/64:128
  - v computed in natural [T, D] layout (bf16 matmuls, full accuracy), then
    split into a compensated fp8 pair: v8 = fp8(v), dv8 = fp8(v - v8), stored
    in vaug [128 kpos, TT, HC, 2, 128] (subtile stride 128 required by the
    dual-fp8 ldweights ISA check); col 64 of the v8 subtile is 1.0 so PV also
    produces the softmax denominator
  - S^T tiles: TWO concurrent row-tiled bf16 matmuls (contraction 64 each,
    array rows 0:63 / 64:127) into one psS tile [128,2,512]
  - P~ = exp(S^T/8) on ScalarE directly to fp8e4, one instruction per psS
    tile (both heads); Scalar runs NOTHING else - exp is the critical path;
    diagonal 128x128 blocks masked with an upper-triangular fp8 mask on DVE
  - PV: one DoubleRow matmul per (k-tile, head): stationary (v8,dv8) pair
    [128,2,66], moving P~ broadcast to both subtiles via a stride-0 dim
    (verified on hw) -> y = p8*(v8+dv8) ~= p8*v at half PE cost with fp8-p8
    accuracy only
  - attention runs in four 512-wide column passes; the output projection for
    pass c-1 overlaps the attention of pass c
  - normalize: DVE reciprocal of denom row, gpsimd partition-broadcast,
    DVE multiply into yT (bf16)
  - proj: out[tt] = yT_tile^T . wp (bf16) accumulated over NP k-tiles
"""

import numpy as np

P = 128


def _bf16_np():
    import ml_dtypes
    return ml_dtypes.bfloat16


def _f8_np():
    import ml_dtypes
    return ml_dtypes.float8_e4m3


WS = 32.0  # host-side scale on wq/wk before fp8 quantization


def build_program(T=2048, C=1024, HC=8, D=64, num_devices=8, trn="TRN2"):
    import concourse.mybir as mybir
    import concourse.tile as tile
    from concourse import bacc
    from concourse.masks import make_upper_triangular

    W = 512          # matmul moving-dim chunk (PSUM bank)
    KC = C // P      # contraction tiles over C (8)
    KP = KC // 2     # fp8 DoubleRow contraction pair-tiles (4)
    CO = HC * D      # this core's qkv channel block (512)
    NP = CO // P     # head pairs (4)
    TT = T // P      # k tiles (16)
    MV = 66          # PV out rows: 64 v-dims + denom + 1 pad
    VS = 128         # vaug subtile stride (dual-fp8 ldweights alignment)
    dt32 = mybir.dt.float32
    bf16 = mybir.dt.bfloat16
    fp8 = mybir.dt.float8e4
    ActF = mybir.ActivationFunctionType
    Alu = mybir.AluOpType
    DR = mybir.MatmulPerfMode.DoubleRow
    scale = 1.0 / float(np.sqrt(D))

    nc = bacc.Bacc(trn, target_bir_lowering=False, debug=False,
                   enable_asserts=False, num_devices=num_devices)

    xt_d = nc.dram_tensor("xt", [C, T], bf16, kind="ExternalInput")
    xt8_d = nc.dram_tensor("xt8", [C, T], fp8, kind="ExternalInput")
    wq_d = nc.dram_tensor("wq", [C, CO], fp8, kind="ExternalInput")
    wk_d = nc.dram_tensor("wk", [C, CO], fp8, kind="ExternalInput")
    wv_d = nc.dram_tensor("wv", [C, CO], bf16, kind="ExternalInput")
    bq_d = nc.dram_tensor("bq", [P, NP], dt32, kind="ExternalInput")
    bvb_d = nc.dram_tensor("bvb", [P, CO], dt32, kind="ExternalInput")
    wp_d = nc.dram_tensor("wp", [CO, C], bf16, kind="ExternalInput")
    out_d = nc.dram_tensor("out", [T, C], bf16, kind="ExternalOutput")
    lsc_d = nc.dram_tensor("lsc", [HC, T], dt32)
    lsc2_d = nc.dram_tensor("lsc2", [HC, T], dt32)

    with tile.TileContext(nc) as tc:
        with tc.tile_pool(name="const", bufs=1) as cpool, \
             tc.tile_pool(name="pers", bufs=1) as pers:
            tri2 = cpool.tile([P, 2, P], fp8)
            make_upper_triangular(nc, tri2[:, 0, :], val=1.0, diag=True)
            make_upper_triangular(nc, tri2[:, 1, :], val=1.0, diag=True)
            bq_sb = cpool.tile([P, NP], dt32)
            bvb_sb = cpool.tile([P, CO], dt32)
            wpsb = cpool.tile([P, NP, C], bf16)

            qT = pers.tile([P, NP, T], bf16, tag="qT")
            kT = pers.tile([P, NP, T], bf16, tag="kT")
            vaug = pers.tile([P, TT, HC, 2, VS], fp8, tag="vaug")
            yT = pers.tile([P, NP, T], bf16, tag="yT")
            # denominator ones column + pads (cols 66:128 are never read)
            nc.vector.memset(vaug[:, :, :, 0, D:D + 1], 1.0)
            nc.vector.memset(vaug[:, :, :, 0, D + 1:MV], 0.0)
            nc.vector.memset(vaug[:, :, :, 1, D:MV], 0.0)

            # psS and ptpool live across stage B and attention: the first
            # column-pass's S tiles are computed interleaved with q/k so the
            # Scalar engine's exp stream starts a few us into the kernel.
            from contextlib import ExitStack
            outer = ExitStack()
            xpool = outer.enter_context(tc.tile_pool(name="xtp", bufs=KC))
            x8pool = outer.enter_context(tc.tile_pool(name="x8p",
                                                      bufs=KP * NP))
            # separate pools per tile size: a shared pool charges every buf
            # at the max tile size
            wpool = outer.enter_context(tc.tile_pool(name="wv_in", bufs=KC))
            wqkpool = outer.enter_context(tc.tile_pool(name="wqk_in",
                                                       bufs=2 * KP * NP))
            vfpool = outer.enter_context(tc.tile_pool(name="vfp", bufs=2))
            ptpool = outer.enter_context(tc.tile_pool(name="ptp", bufs=21))
            psS = outer.enter_context(
                tc.tile_pool(name="psS", bufs=2, space="PSUM"))

            def emit_s(m, j, plo, phi):
                """Paired S^T + exp for heads (2m, 2m+1), k-tile j,
                columns [max(jb,plo), phi). Returns the pt pair tile."""
                jb = j * P
                qlo = max(jb, plo)
                w = phi - qlo
                pt = ptpool.tile([P, 2, W], fp8, tag="pt")
                sps = psS.tile([P, 2, W], dt32, tag="s")
                nc.tensor.matmul(
                    sps[:, 0, 0:w],
                    kT[0:D, m, jb:jb + P],
                    qT[0:D, m, qlo:phi],
                    start=True, stop=True, skip_group_check=True)
                nc.tensor.matmul(
                    sps[:, 1, 0:w],
                    kT[D:P, m, jb:jb + P],
                    qT[D:P, m, qlo:phi],
                    start=True, stop=True, skip_group_check=True)
                nc.scalar.activation(
                    pt[:, :, 0:w], sps[:, :, 0:w], ActF.Exp, scale=scale)
                if jb >= plo:  # diagonal block lives in this pass
                    nc.vector.tensor_mul(pt[:, :, 0:P], pt[:, :, 0:P],
                                         tri2[:])
                return pt

            # ---- stage B: q/k projections + v tiles 0..3 + pass-0 S ------
            pass0_pts = [[] for _ in range(NP)]
            with nc.named_scope("qkv"), \
                 tc.tile_pool(name="psB", bufs=4, space="PSUM") as psB:
                xt_view = xt_d.ap().rearrange("(kc p) t -> kc p t", p=P)
                x8_view = xt8_d.ap().rearrange(
                    "(kk two p) t -> kk p two t", two=2, p=P)
                # all load dma_starts are issued before the first exp, so the
                # scalar queue is safe to use for issuance here (the dma
                # transfer itself is async on the dma engines)
                dmae = [nc.sync, nc.scalar, nc.gpsimd]
                di = [0]

                def dma(dst, src):
                    dmae[di[0] % 3].dma_start(dst, src)
                    di[0] += 1

                # fp8 pair tiles first: q matmuls can start ~2MB in
                x8t = {}
                wq_t = {}
                wk_t = {}
                wq_view = wq_d.ap().rearrange(
                    "(kk two p) n -> kk p two n", two=2, p=P)
                wk_view = wk_d.ap().rearrange(
                    "(kk two p) n -> kk p two n", two=2, p=P)
                for kk in range(KP):
                    for m in range(NP):
                        wt = wqkpool.tile([P, 2, P], fp8, tag="wqk",
                                          name=f"wq_{kk}_{m}")
                        dma(wt[:], wq_view[kk][:, :, m * P:(m + 1) * P])
                        wq_t[kk, m] = wt
                    for tq in range(NP):
                        xt8c = x8pool.tile([P, 2, W], fp8, tag="x8",
                                           name=f"x8_{kk}_{tq}")
                        dma(xt8c[:], x8_view[kk][:, :, tq * W:(tq + 1) * W])
                        x8t[kk, tq] = xt8c
                nc.sync.dma_start(bq_sb[:], bq_d.ap())
                for kk in range(KP):
                    for m in range(NP):
                        wt = wqkpool.tile([P, 2, P], fp8, tag="wqk",
                                          name=f"wk_{kk}_{m}")
                        dma(wt[:], wk_view[kk][:, :, m * P:(m + 1) * P])
                        wk_t[kk, m] = wt
                # bf16 x + wv for the v path; wp last
                xts = []
                wv_t = []
                for kc in range(KC):
                    xtc = xpool.tile([P, T], bf16, tag="xt")
                    dma(xtc[:], xt_view[kc])
                    xts.append(xtc)
                    wt = wpool.tile([P, CO], bf16, tag="w", name=f"wv_{kc}")
                    dma(wt[:], wv_d.ap().rearrange(
                        "(kc p) n -> kc p n", p=P)[kc])
                    wv_t.append(wt)
                nc.gpsimd.dma_start(bvb_sb[:], bvb_d.ap())
                nc.sync.dma_start(
                    wpsb[:], wp_d.ap().rearrange("(kt p) n -> p kt n", p=P))
                bvb_v = bvb_sb[:].rearrange("p (h d) -> p h d", d=D)

                def qk_m(w_tiles, dst, m, bias):
                    pss = {}
                    for tq in range(4):
                        pss[tq] = psB.tile([P, W], dt32, tag="psB",
                                           name=f"psB_{tq}")
                    for kk in range(KP):
                        for tq in range(4):
                            nc.tensor.matmul(
                                pss[tq][:],
                                w_tiles[kk, m][:],
                                x8t[kk, tq][:],
                                start=(kk == 0), stop=(kk == KP - 1),
                                perf_mode=DR, skip_group_check=True)
                    for tq in range(4):
                        if bias is not None:
                            nc.vector.tensor_scalar(
                                out=dst[:, m, tq * W:(tq + 1) * W],
                                in0=pss[tq][:],
                                scalar1=1.0 / WS, scalar2=bias,
                                op0=Alu.mult, op1=Alu.add)
                        else:
                            nc.vector.tensor_scalar(
                                out=dst[:, m, tq * W:(tq + 1) * W],
                                in0=pss[tq][:],
                                scalar1=1.0 / WS, scalar2=None,
                                op0=Alu.mult)

                def emit_v_evac(tt, ps):
                    vf = ps[:].rearrange("p (h d) -> p h d", d=D)
                    vfull = vfpool.tile([P, HC, D], bf16, tag="vf",
                                        name="vfull")
                    nc.vector.scalar_tensor_tensor(
                        out=vfull[:], in0=vf, scalar=1.0, in1=bvb_v,
                        op0=Alu.mult, op1=Alu.add)
                    nc.gpsimd.tensor_copy(
                        vaug[:, tt, :, 0, 0:D], vfull[:])
                    nc.gpsimd.tensor_tensor(
                        out=vaug[:, tt, :, 1, 0:D],
                        in0=vfull[:], in1=vaug[:, tt, :, 0, 0:D],
                        op=Alu.subtract)

                def emit_v_b(tt):
                    ps = psB.tile([P, CO], dt32, tag="psB", name="vps")
                    for kc in range(KC):
                        nc.tensor.matmul(
                            ps[:],
                            xts[kc][:, tt * P:(tt + 1) * P],
                            wv_t[kc][:],
                            start=(kc == 0), stop=(kc == KC - 1))
                    emit_v_evac(tt, ps)

                # v(m) fills the PE while m's q-group PSUM evacuations
                # drain; S(m) follows k(m) immediately so the exp stream
                # starts a few us into the kernel
                pass1_pre = []
                for m in range(NP):
                    qk_m(wq_t, qT, m, bq_sb[:, m:m + 1])
                    emit_v_b(m)
                    qk_m(wk_t, kT, m, None)
                    for j in range(4):
                        pass0_pts[m].append((j, emit_s(m, j, 0, W)))
                # a few pass-1 chunks keep exp fed through the last q/k
                # groups (pt slots: 16 pass-0 + 4 here, pool is 21)
                for j in range(4):
                    pass1_pre.append((j, emit_s(0, j, W, 2 * W)))

            # ------- stage C/E: attention + projection --------------------
            # four 512-wide column sub-passes; pass c consumes k-tiles
            # j <= 4c+3; the previous pass's projection tiles are
            # interleaved with the next pass so proj overlaps attention.
            with nc.named_scope("attn"), \
                 tc.tile_pool(name="nrm", bufs=4) as nrmpool, \
                 tc.tile_pool(name="ost", bufs=2) as opool, \
                 tc.tile_pool(name="psY", bufs=3, space="PSUM") as psY, \
                 tc.tile_pool(name="psO", bufs=1, space="PSUM") as psO:

                # v-tiles and proj-tiles are emitted as small MM pieces fed
                # one-per-S-chunk between exp stages, so their bursts never
                # starve the Scalar exp stream
                def make_v_pieces(tt):
                    st = {}

                    def p1():
                        st["ps"] = psO.tile([P, CO], dt32, tag="o",
                                            name="vps")
                        for kc in range(KC // 2):
                            nc.tensor.matmul(
                                st["ps"][:],
                                xts[kc][:, tt * P:(tt + 1) * P],
                                wv_t[kc][:],
                                start=(kc == 0), stop=False)

                    def p2():
                        for kc in range(KC // 2, KC):
                            nc.tensor.matmul(
                                st["ps"][:],
                                xts[kc][:, tt * P:(tt + 1) * P],
                                wv_t[kc][:],
                                start=False, stop=(kc == KC - 1))
                        emit_v_evac(tt, st["ps"])

                    return [p1, p2]

                def make_proj_pieces(tt):
                    st = {}

                    def mk(nn, half):
                        def piece():
                            if half == 0:
                                st[nn] = psO.tile([P, W], dt32, tag="o",
                                                  name=f"po{nn}")
                                if nn == 0:
                                    st["ot"] = opool.tile([P, C], bf16,
                                                          tag="ot",
                                                          name="ot")
                                kts = (0, 1)
                            else:
                                kts = (2, 3)
                            for kt in kts:
                                nc.tensor.matmul(
                                    st[nn][:],
                                    yT[:, kt, tt * P:(tt + 1) * P],
                                    wpsb[:, kt, nn * W:(nn + 1) * W],
                                    start=(kt == 0), stop=(kt == NP - 1),
                                    skip_group_check=True)
                            if half == 1:
                                nc.vector.tensor_copy(
                                    st["ot"][:, nn * W:(nn + 1) * W],
                                    st[nn][:])
                                if nn == 1:
                                    [nc.sync, nc.gpsimd][tt % 2].dma_start(
                                        out_d.ap()[tt * P:(tt + 1) * P, :],
                                        st["ot"][:])
                        return piece

                    return [mk(0, 0), mk(0, 1), mk(1, 0), mk(1, 1)]

                fillers = []  # entries: (v_tt_or_minus1, fn)

                def pump(n=1):
                    for _ in range(n):
                        if fillers:
                            fillers.pop(0)[1]()

                def pump_v_upto(tt):
                    while fillers and 0 <= fillers[0][0] <= tt:
                        fillers.pop(0)[1]()

                def emit_pv(h, j, pt, yt, plo, phi, jmax):
                    jb = j * P
                    qlo = max(jb, plo)
                    w = phi - qlo
                    nc.tensor.matmul(
                        yt[:, qlo - plo:phi - plo],
                        vaug[:, j, h, :, 0:MV],
                        pt[:, h % 2, 0:w].rearrange(
                            "p w -> p () w").broadcast_to([P, 2, w]),
                        start=(j == 0), stop=(j == jmax),
                        perf_mode=DR, skip_group_check=True)

                # finish is a 3-stage pipeline across head-pairs so no DVE op
                # ever waits at the head of the queue on an in-flight DMA:
                #   front: evacuate yt PSUM + kick the denom-row fold DMAs
                #   mid (a pair later): reciprocal + kick the broadcast DMAs
                #   back (another pair later): normalize-multiply into yT
                fin_q1, fin_q2 = [], []

                def finish_front(h, c, yt, plo, phi):
                    ys = nrmpool.tile([D + 1, W], dt32, tag="ys")
                    nc.vector.tensor_copy(ys[:], yt[0:D + 1, :])
                    nc.sync.dma_start(
                        lsc_d.ap()[h, plo:phi].rearrange("(o t) -> o t", o=1),
                        ys[D:D + 1, :])
                    dn = nrmpool.tile([P, W // P], dt32, tag="dn")
                    nc.gpsimd.dma_start(
                        dn[:],
                        lsc_d.ap()[h, plo:phi].rearrange("(p c) -> p c", p=P))
                    fin_q1.append((h, ys, dn, plo, phi))

                def finish_mid(st):
                    h, ys, dn, plo, phi = st
                    nc.vector.reciprocal(dn[:], dn[:])
                    nc.gpsimd.dma_start(
                        lsc2_d.ap()[h, plo:phi].rearrange("(p c) -> p c", p=P),
                        dn[:])
                    bc = nrmpool.tile([D, W], dt32, tag="bc")
                    nc.sync.dma_start(
                        bc[:],
                        lsc2_d.ap()[h, plo:phi].rearrange(
                            "(o t) -> o t", o=1).broadcast_to([D, W]))
                    fin_q2.append((h, ys, bc, plo, phi))

                def finish_back(st):
                    h, ys, bc, plo, phi = st
                    r0 = (h % 2) * D
                    nc.vector.tensor_mul(
                        yT[r0:r0 + D, h // 2, plo:phi], ys[0:D, :], bc[:])

                def finish_step():
                    while len(fin_q1) > 2:
                        finish_mid(fin_q1.pop(0))
                    while len(fin_q2) > 2:
                        finish_back(fin_q2.pop(0))

                def finish_flush():
                    while fin_q1:
                        finish_mid(fin_q1.pop(0))
                    while fin_q2:
                        finish_back(fin_q2.pop(0))

                def finish_fast(h, yt, plo, phi):
                    """DMA-free normalize (gpsimd broadcast + fast DVE
                    reciprocal) — low latency, for the last column pass."""
                    drow = nrmpool.tile([1, W], dt32, tag="drow")
                    nc.vector.tensor_copy(drow[:], yt[D:D + 1, :])
                    ys = nrmpool.tile([D + 1, W], dt32, tag="ys")
                    nc.vector.tensor_copy(ys[0:D, :], yt[0:D, :])
                    bc = nrmpool.tile([D, W], dt32, tag="bc")
                    nc.gpsimd.partition_broadcast(bc[:], drow[:], channels=D)
                    rec = nrmpool.tile([D, W], dt32, tag="bc", name="rec")
                    nc.vector.reciprocal_approx_fast(out=rec[:], in_=bc[:])
                    r0 = (h % 2) * D
                    nc.vector.tensor_mul(
                        yT[r0:r0 + D, h // 2, plo:phi], ys[0:D, :], rec[:])

                def emit_proj_mms(tt, pos, k0, k1):
                    for kt in range(k0, k1):
                        for nn in range(2):
                            nc.tensor.matmul(
                                pos[nn][:],
                                yT[:, kt, tt * P:(tt + 1) * P],
                                wpsb[:, kt, nn * W:(nn + 1) * W],
                                start=(kt == 0), stop=(kt == NP - 1),
                                skip_group_check=True)

                def emit_proj_done(tt, pos):
                    ot = opool.tile([P, C], bf16, tag="ot")
                    for nn in range(2):
                        nc.vector.tensor_copy(
                            ot[:, nn * W:(nn + 1) * W], pos[nn][:])
                    [nc.sync, nc.gpsimd][tt % 2].dma_start(
                        out_d.ap()[tt * P:(tt + 1) * P, :], ot[:])

                def emit_proj_tt(tt):
                    # kernel-tail only: S traffic is done, reuse a psS slot
                    pop = psS.tile([P, 2, W], dt32, tag="s", name="pop")
                    pos = [pop[:, 0, :], pop[:, 1, :]]
                    emit_proj_mms(tt, pos, 0, NP)
                    emit_proj_done(tt, pos)

                # Global S-emission cursor kept ~14 chunks ahead of PV
                # consumption: the exp stream never starves at pair or pass
                # boundaries. Pass 0 S tiles were pre-built in stage B.
                sq = [(c2, m2, j2) for c2 in range(1, 4) for m2 in range(NP)
                      for j2 in range(4 * c2 + 4)
                      if not (c2 == 1 and m2 == 0 and j2 < 4)]
                sq_i = 0
                pts_q = {(0, m2): list(pass0_pts[m2]) for m2 in range(NP)}
                pts_q[1, 0] = list(pass1_pre)
                state = {"ahead": sum(len(v) for v in pts_q.values())}

                def pull_s():
                    c2, m2, j2 = sq[sq_i]
                    pt = emit_s(m2, j2, c2 * W, (c2 + 1) * W)
                    pts_q.setdefault((c2, m2), []).append((j2, pt))
                    state["ahead"] += 1

                # proj tiles of pass c-1 interleave with pass c, placed so
                # every yT write they read is already emitted
                proj_hooks = {}
                for c2 in range(1, 4):
                    proj_hooks[c2, 1] = [4 * (c2 - 1)]
                    proj_hooks[c2, 2] = [4 * (c2 - 1) + 1]
                    proj_hooks[c2, 3] = [4 * (c2 - 1) + 2, 4 * (c2 - 1) + 3]

                # all v pieces are ready to run from the start (they only
                # need xts/wv_t): enqueue them all up front so they trickle
                # out between S chunks instead of bunching at pass boundaries
                for tt in range(4, TT):
                    fillers.extend(
                        (tt, f) for f in make_v_pieces(tt))

                for c in range(4):
                    plo, phi = c * W, (c + 1) * W
                    jmax = 4 * c + 3
                    # v-tiles this pass reads must be emitted before its
                    # first PV; later fillers keep trickling
                    pump_v_upto(jmax)
                    for m in range(NP):
                        hA, hB = 2 * m, 2 * m + 1
                        ytA = psY.tile([MV, W], dt32, tag="yt", name="ytA")
                        ytB = psY.tile([MV, W], dt32, tag="yt", name="ytB")
                        while (len(pts_q.get((c, m), [])) < jmax + 1
                               and sq_i < len(sq)):
                            pull_s()
                            sq_i += 1
                            pump()
                        for pj, ppt in pts_q.pop((c, m)):
                            emit_pv(hA, pj, ppt, ytA, plo, phi, jmax)
                            emit_pv(hB, pj, ppt, ytB, plo, phi, jmax)
                            state["ahead"] -= 1
                            while state["ahead"] < 16 and sq_i < len(sq):
                                pull_s()
                                sq_i += 1
                                pump()
                        if c < 3 or m == 0:
                            finish_front(hA, c, ytA, plo, phi)
                            finish_front(hB, c, ytB, plo, phi)
                            finish_step()
                            for tt in proj_hooks.get((c, m), []):
                                fillers.extend(
                                    (-1, f) for f in make_proj_pieces(tt))
                        elif m < NP - 1:
                            # last pass: low-latency normalize, no DMA bounce
                            finish_fast(hA, ytA, plo, phi)
                            finish_fast(hB, ytB, plo, phi)
                            if m == 1:
                                finish_flush()
                            for tt in proj_hooks.get((c, m), []):
                                fillers.extend(
                                    (-1, f) for f in make_proj_pieces(tt))
                        else:
                            for tt in proj_hooks.get((c, m), []):
                                fillers.extend(
                                    (-1, f) for f in make_proj_pieces(tt))
                            pump(len(fillers))
                            # all other pass-3 yT writes are emitted: the
                            # final proj's first 3 kt keep the PE warm while
                            # the last normalize chain drains
                            pop12 = psS.tile([P, 2, W], dt32, tag="s",
                                             name="pop12")
                            pos12 = [pop12[:, 0, :], pop12[:, 1, :]]
                            emit_proj_mms(12, pos12, 0, NP - 1)
                            finish_fast(hA, ytA, plo, phi)
                            finish_fast(hB, ytB, plo, phi)
                            emit_proj_mms(12, pos12, NP - 1, NP)
                            emit_proj_done(12, pos12)
                for tt in range(13, TT):
                    emit_proj_tt(tt)
            outer.close()

    nc.compile()
    return nc


def make_core_inputs(x, W_attn, b_attn, W_proj, n_cores=8, HC=8, D=64):
    """Host-side sharding: per-core input dicts."""
    B, T, C = x.shape
    CO = HC * D
    NP = CO // P
    bf = _bf16_np()
    f8 = _f8_np()
    in_maps = []
    for c in range(n_cores):
        b = c // (n_cores // B)
        h0 = (c % (n_cores // B)) * HC
        lo = h0 * D
        bq = b_attn[lo:lo + CO]
        bv = b_attn[2 * C + lo:2 * C + lo + CO]
        xtb = np.ascontiguousarray(x[b].T)
        in_maps.append({
            "xt": xtb.astype(bf),
            "xt8": xtb.astype(f8),
            "wq": np.ascontiguousarray(
                W_attn[:, lo:lo + CO] * WS).astype(f8),
            "wk": np.ascontiguousarray(
                W_attn[:, C + lo:C + lo + CO] * WS).astype(f8),
            "wv": np.ascontiguousarray(
                W_attn[:, 2 * C + lo:2 * C + lo + CO]).astype(bf),
            "bq": np.ascontiguousarray(bq.reshape(NP, P).T),
            "bvb": np.tile(bv[None, :], (P, 1)),
            "wp": np.ascontiguousarray(W_proj[lo:lo + CO, :]).astype(bf),
        })
    return in_maps


_CACHE = {}


def _get_program():
    if "nc" not in _CACHE:
        _CACHE["nc"] = build_program()
    return _CACHE["nc"]


def run_on_cores(x, W_attn, b_attn, W_proj, b_proj, trace=False):
    """Returns (full output [B,T,C], BassKernelResults)."""
    from concourse.bass_utils import run_bass_kernel_spmd

    x = np.asarray(x, np.float32)
    W_attn = np.asarray(W_attn, np.float32)
    b_attn = np.asarray(b_attn, np.float32)
    W_proj = np.asarray(W_proj, np.float32)
    b_proj = np.asarray(b_proj, np.float32)

    nc = _get_program()
    in_maps = make_core_inputs(x, W_attn, b_attn, W_proj)
    res = run_bass_kernel_spmd(nc, in_maps, core_ids=list(range(8)), trace=trace)
    B, T, C = x.shape
    out = np.empty((B, T, C), np.float32)
    for b in range(B):
        out[b] = (res.results[2 * b]["out"].astype(np.float32)
                  + res.results[2 * b + 1]["out"].astype(np.float32)
                  + b_proj[None, :])
    return out, res


def kernel(x, W_attn, b_attn, W_proj, b_proj):
    out, _ = run_on_cores(x, W_attn, b_attn, W_proj, b_proj, trace=False)
    return out


# revision 7
# speedup vs baseline: 1.1391x; 1.1391x over previous
"""Causal self-attention (B=4, T=2048, C=1024, H=16) on 8 trn2 NeuronCores.

Sharding: core c -> batch b = c//2, heads h0 = (c%2)*8 .. h0+8 (tensor
parallel over heads: c_attn columns / c_proj rows split). Each core computes a
partial projection output [T, C] in bf16; the host sums the two partials per
batch and adds b_proj.

Device-side dataflow:
  - host passes x[b] pre-transposed twice: xt [C, T] bf16 (v path) and
    xt8 [C, T] fp8e4 (q/k path); wq/wk are fp8e4 scaled by 32
  - q/k projections: fp8 DoubleRow matmuls folding two 128-row contraction
    tiles per pass (2x PE throughput); PSUM evacuated on DVE tensor_scalar
    (scale 1/32 + bq bias for q; bk is DROPPED entirely - a per-query-constant
    logit shift cancels in softmax). Softmax is tolerant to the ~2% fp8 q/k
    noise; v and the output path stay bf16.
  - stage B is ordered for exp latency: q/k columns 0:512 of each head pair
    are projected first and their pass-0 S tiles emitted immediately, so the
    Scalar exp stream starts ~10us into the kernel
  - qT, kT [128, NP, T] bf16: head pair 2m/2m+1 stacked on partitions
    0:64/64:128
  - v computed in natural [T, D] layout (bf16) into vaug [128 kpos, TT, HC,
    MV] with a ones column at col D so the PV matmul also produces the
    softmax denominator
  - S^T tiles: TWO concurrent row-tiled bf16 matmuls (contraction 64 each,
    array rows 0:63 / 64:127) into one psS tile [128,2,512]
  - P~ = exp(S^T/8) on ScalarE, one instruction per psS tile (both heads);
    Scalar runs nothing else after the load phase - exp is the critical path;
    diagonal 128x128 blocks masked with an upper-triangular mask on DVE
  - attention runs in four 512-wide column passes; the output projection for
    pass c-1 overlaps the attention of pass c
  - yT_aug [MV, 1024] += vaug_tile^T . P~ accumulated in PSUM over k-tiles
  - normalize: DVE reciprocal of denom row, gpsimd partition-broadcast,
    DVE multiply into yT (bf16)
  - proj: out[tt] = yT_tile^T . wp accumulated over NP k-tiles, bf16 to DRAM
"""

import numpy as np

P = 128


def _bf16_np():
    import ml_dtypes
    return ml_dtypes.bfloat16


def _f8_np():
    import ml_dtypes
    return ml_dtypes.float8_e4m3


WS = 32.0  # host-side scale on wq/wk before fp8 quantization


def build_program(T=2048, C=1024, HC=8, D=64, num_devices=8, trn="TRN2"):
    import concourse.mybir as mybir
    import concourse.tile as tile
    from concourse import bacc
    from concourse.masks import make_upper_triangular

    W = 512          # matmul moving-dim chunk (PSUM bank)
    KC = C // P      # contraction tiles over C (8)
    KP = KC // 2     # fp8 DoubleRow contraction pair-tiles (4)
    CO = HC * D      # this core's qkv channel block (512)
    NP = CO // P     # head pairs (4)
    TT = T // P      # k tiles (16)
    TH = T // 2      # column-pass width (1024)
    MV = 66          # PV stationary cols: 64 v-dims + ones + 1 pad
    dt32 = mybir.dt.float32
    bf16 = mybir.dt.bfloat16
    fp8 = mybir.dt.float8e4
    ActF = mybir.ActivationFunctionType
    Alu = mybir.AluOpType
    DR = mybir.MatmulPerfMode.DoubleRow
    scale = 1.0 / float(np.sqrt(D))

    nc = bacc.Bacc(trn, target_bir_lowering=False, debug=False,
                   enable_asserts=False, num_devices=num_devices)

    xt_d = nc.dram_tensor("xt", [C, T], bf16, kind="ExternalInput")
    xt8_d = nc.dram_tensor("xt8", [C, T], fp8, kind="ExternalInput")
    wq_d = nc.dram_tensor("wq", [C, CO], fp8, kind="ExternalInput")
    wk_d = nc.dram_tensor("wk", [C, CO], fp8, kind="ExternalInput")
    wv_d = nc.dram_tensor("wv", [C, CO], bf16, kind="ExternalInput")
    bq_d = nc.dram_tensor("bq", [P, NP], dt32, kind="ExternalInput")
    bvb_d = nc.dram_tensor("bvb", [P, CO], dt32, kind="ExternalInput")
    wp_d = nc.dram_tensor("wp", [CO, C], bf16, kind="ExternalInput")
    out_d = nc.dram_tensor("out", [T, C], bf16, kind="ExternalOutput")
    lsc_d = nc.dram_tensor("lsc", [HC, T], dt32)
    lsc2_d = nc.dram_tensor("lsc2", [HC, T], dt32)

    with tile.TileContext(nc) as tc:
        with tc.tile_pool(name="const", bufs=1) as cpool, \
             tc.tile_pool(name="pers", bufs=1) as pers:
            tri2 = cpool.tile([P, 2, P], bf16)
            make_upper_triangular(nc, tri2[:, 0, :], val=1.0, diag=True)
            make_upper_triangular(nc, tri2[:, 1, :], val=1.0, diag=True)
            bq_sb = cpool.tile([P, NP], dt32)
            bvb_sb = cpool.tile([P, CO], dt32)
            wpsb = cpool.tile([P, NP, C], bf16)

            qT = pers.tile([P, NP, T], bf16, tag="qT")
            kT = pers.tile([P, NP, T], bf16, tag="kT")
            vaug = pers.tile([P, TT, HC, MV], bf16, tag="vaug")
            yT = pers.tile([P, NP, T], bf16, tag="yT")
            nc.vector.memset(vaug[:, :, :, D:D + 1], 1.0)

            # psS and ptpool live across stage B and attention: pass-0/1 S
            # tiles are computed interleaved with q/k so the Scalar engine's
            # exp stream starts ~10us into the kernel.
            from contextlib import ExitStack
            outer = ExitStack()
            xpool = outer.enter_context(tc.tile_pool(name="xtp", bufs=KC))
            x8pool = outer.enter_context(tc.tile_pool(name="x8p",
                                                      bufs=KP * NP))
            wpool = outer.enter_context(tc.tile_pool(name="wv_in", bufs=KC))
            wqkpool = outer.enter_context(tc.tile_pool(name="wqk_in",
                                                       bufs=2 * KP * NP))
            ptpool = outer.enter_context(tc.tile_pool(name="ptp", bufs=20))
            psS = outer.enter_context(
                tc.tile_pool(name="psS", bufs=2, space="PSUM"))

            def emit_s(m, j, plo, phi):
                """Paired S^T + exp for heads (2m, 2m+1), k-tile j,
                columns [max(jb,plo), phi). Returns the pt pair tile."""
                jb = j * P
                qlo = max(jb, plo)
                w = phi - qlo
                pt = ptpool.tile([P, 2, W], bf16, tag="pt")
                sps = psS.tile([P, 2, W], dt32, tag="s")
                nc.tensor.matmul(
                    sps[:, 0, 0:w],
                    kT[0:D, m, jb:jb + P],
                    qT[0:D, m, qlo:phi],
                    start=True, stop=True, skip_group_check=True)
                nc.tensor.matmul(
                    sps[:, 1, 0:w],
                    kT[D:P, m, jb:jb + P],
                    qT[D:P, m, qlo:phi],
                    start=True, stop=True, skip_group_check=True)
                nc.scalar.activation(
                    pt[:, :, 0:w], sps[:, :, 0:w], ActF.Exp, scale=scale)
                if jb >= plo:  # diagonal block lives in this pass
                    nc.vector.tensor_mul(pt[:, :, 0:P], pt[:, :, 0:P],
                                         tri2[:])
                return pt

            # ---- stage B: q/k projections + v tiles 0..3 + pass-0 S ------
            pass0_pts = [[] for _ in range(NP)]
            with nc.named_scope("qkv"), \
                 tc.tile_pool(name="psB", bufs=4, space="PSUM") as psB:
                xt_view = xt_d.ap().rearrange("(kc p) t -> kc p t", p=P)
                x8_view = xt8_d.ap().rearrange(
                    "(kk two p) t -> kk p two t", two=2, p=P)
                wq_view = wq_d.ap().rearrange(
                    "(kk two p) n -> kk p two n", two=2, p=P)
                wk_view = wk_d.ap().rearrange(
                    "(kk two p) n -> kk p two n", two=2, p=P)
                # all load dma_starts precede the first exp in program order,
                # so the scalar queue is safe for issuance here
                dmae = [nc.sync, nc.scalar, nc.gpsimd]
                di = [0]

                def dma(dst, src):
                    dmae[di[0] % 3].dma_start(dst, src)
                    di[0] += 1

                x8t = {}
                wq_t = {}
                wk_t = {}

                def load_x8(tq, kk):
                    t = x8pool.tile([P, 2, W], fp8, tag="x8",
                                    name=f"x8_{kk}_{tq}")
                    dma(t[:], x8_view[kk][:, :, tq * W:(tq + 1) * W])
                    x8t[kk, tq] = t

                def load_wqk(view, store, nm, m, kk):
                    t = wqkpool.tile([P, 2, P], fp8, tag="wqk",
                                     name=f"{nm}_{kk}_{m}")
                    dma(t[:], view[kk][:, :, m * P:(m + 1) * P])
                    store[kk, m] = t

                # critical loads first: everything the first S chunks need
                for kk in range(KP):
                    load_x8(0, kk)
                    load_wqk(wq_view, wq_t, "wq", 0, kk)
                    load_wqk(wk_view, wk_t, "wk", 0, kk)
                nc.sync.dma_start(bq_sb[:], bq_d.ap())
                for m in range(1, NP):
                    for kk in range(KP):
                        load_wqk(wq_view, wq_t, "wq", m, kk)
                        load_wqk(wk_view, wk_t, "wk", m, kk)
                for tq in range(1, NP):
                    for kk in range(KP):
                        load_x8(tq, kk)
                xts = []
                wv_t = []
                for kc in range(KC):
                    xtc = xpool.tile([P, T], bf16, tag="xt")
                    dma(xtc[:], xt_view[kc])
                    xts.append(xtc)
                    wt = wpool.tile([P, CO], bf16, tag="w", name=f"wv_{kc}")
                    dma(wt[:], wv_d.ap().rearrange(
                        "(kc p) n -> kc p n", p=P)[kc])
                    wv_t.append(wt)
                nc.gpsimd.dma_start(bvb_sb[:], bvb_d.ap())
                nc.sync.dma_start(
                    wpsb[:], wp_d.ap().rearrange("(kt p) n -> p kt n", p=P))
                bvb_v = bvb_sb[:].rearrange("p (h d) -> p h d", d=D)

                def qk_tq(w_tiles, dst, m, tq, bias):
                    ps = psB.tile([P, W], dt32, tag="psB", name=f"psB_{tq}")
                    for kk in range(KP):
                        nc.tensor.matmul(
                            ps[:],
                            w_tiles[kk, m][:],
                            x8t[kk, tq][:],
                            start=(kk == 0), stop=(kk == KP - 1),
                            perf_mode=DR, skip_group_check=True)
                    if bias is not None:
                        nc.vector.tensor_scalar(
                            out=dst[:, m, tq * W:(tq + 1) * W],
                            in0=ps[:],
                            scalar1=1.0 / WS, scalar2=bias,
                            op0=Alu.mult, op1=Alu.add)
                    else:
                        nc.vector.tensor_scalar(
                            out=dst[:, m, tq * W:(tq + 1) * W],
                            in0=ps[:],
                            scalar1=1.0 / WS, scalar2=None,
                            op0=Alu.mult)

                def emit_v_b(tt):
                    ps = psB.tile([P, CO], dt32, tag="psB", name="vps")
                    for kc in range(KC):
                        nc.tensor.matmul(
                            ps[:],
                            xts[kc][:, tt * P:(tt + 1) * P],
                            wv_t[kc][:],
                            start=(kc == 0), stop=(kc == KC - 1))
                    nc.vector.scalar_tensor_tensor(
                        out=vaug[:, tt, :, 0:D],
                        in0=ps[:].rearrange("p (h d) -> p h d", d=D),
                        scalar=1.0, in1=bvb_v,
                        op0=Alu.mult, op1=Alu.add)

                # column block 0:512 of each head pair first, with its pass-0
                # S tiles emitted immediately: exp starts as soon as the fp8
                # pair tiles land. The remaining columns + v tiles follow.
                for m in range(NP):
                    qk_tq(wq_t, qT, m, 0, bq_sb[:, m:m + 1])
                    qk_tq(wk_t, kT, m, 0, None)
                    for j in range(4):
                        pass0_pts[m].append((j, emit_s(m, j, 0, W)))
                pass1_pre = []
                for m in range(NP):
                    for tq in range(1, NP):
                        qk_tq(wq_t, qT, m, tq, bq_sb[:, m:m + 1])
                        qk_tq(wk_t, kT, m, tq, None)
                    if m == 0:
                        # pass-1 chunks keep exp fed through the remaining
                        # q/k groups (pt slots: 16 pass-0 + 4 here, pool 21)
                        for j in range(4):
                            pass1_pre.append((j, emit_s(0, j, W, 2 * W)))
                    emit_v_b(m)

            # ------- stage C/E: attention + projection --------------------
            # four 512-wide column sub-passes; pass c consumes k-tiles
            # j <= 4c+3; the previous pass's projection tiles are
            # interleaved with the next pass so proj overlaps attention.
            with nc.named_scope("attn"), \
                 tc.tile_pool(name="nrm", bufs=4) as nrmpool, \
                 tc.tile_pool(name="ost", bufs=2) as opool, \
                 tc.tile_pool(name="psY", bufs=3, space="PSUM") as psY, \
                 tc.tile_pool(name="psO", bufs=1, space="PSUM") as psO:

                # v-tiles and proj-tiles are emitted as small MM pieces fed
                # one-per-S-chunk between exp stages, so their bursts never
                # starve the Scalar exp stream
                def make_v_pieces(tt):
                    st = {}

                    def p1():
                        st["ps"] = psO.tile([P, CO], dt32, tag="o",
                                            name="vps")
                        for kc in range(KC // 2):
                            nc.tensor.matmul(
                                st["ps"][:],
                                xts[kc][:, tt * P:(tt + 1) * P],
                                wv_t[kc][:],
                                start=(kc == 0), stop=False)

                    def p2():
                        for kc in range(KC // 2, KC):
                            nc.tensor.matmul(
                                st["ps"][:],
                                xts[kc][:, tt * P:(tt + 1) * P],
                                wv_t[kc][:],
                                start=False, stop=(kc == KC - 1))
                        nc.vector.scalar_tensor_tensor(
                            out=vaug[:, tt, :, 0:D],
                            in0=st["ps"][:].rearrange("p (h d) -> p h d", d=D),
                            scalar=1.0, in1=bvb_v,
                            op0=Alu.mult, op1=Alu.add)

                    return [p1, p2]

                def make_proj_pieces(tt):
                    st = {}

                    def mk(nn, half):
                        def piece():
                            if half == 0:
                                st[nn] = psO.tile([P, W], dt32, tag="o",
                                                  name=f"po{nn}")
                                if nn == 0:
                                    st["ot"] = opool.tile([P, C], bf16,
                                                          tag="ot",
                                                          name="ot")
                                kts = (0, 1)
                            else:
                                kts = (2, 3)
                            for kt in kts:
                                nc.tensor.matmul(
                                    st[nn][:],
                                    yT[:, kt, tt * P:(tt + 1) * P],
                                    wpsb[:, kt, nn * W:(nn + 1) * W],
                                    start=(kt == 0), stop=(kt == NP - 1),
                                    skip_group_check=True)
                            if half == 1:
                                nc.vector.tensor_copy(
                                    st["ot"][:, nn * W:(nn + 1) * W],
                                    st[nn][:])
                                if nn == 1:
                                    [nc.sync, nc.gpsimd][tt % 2].dma_start(
                                        out_d.ap()[tt * P:(tt + 1) * P, :],
                                        st["ot"][:])
                        return piece

                    return [mk(0, 0), mk(0, 1), mk(1, 0), mk(1, 1)]

                fillers = []  # entries: (v_tt_or_minus1, fn)

                def pump(n=1):
                    for _ in range(n):
                        if fillers:
                            fillers.pop(0)[1]()

                def pump_v_upto(tt):
                    while fillers and 0 <= fillers[0][0] <= tt:
                        fillers.pop(0)[1]()

                def emit_pv(h, j, pt, yt, plo, phi, jmax):
                    jb = j * P
                    qlo = max(jb, plo)
                    nc.tensor.matmul(
                        yt[:, qlo - plo:phi - plo],
                        vaug[:, j, h, :],
                        pt[:, h % 2, 0:phi - qlo],
                        start=(j == 0), stop=(j == jmax),
                        skip_group_check=True)

                # finish is a 3-stage pipeline across head-pairs so no DVE op
                # ever waits at the head of the queue on an in-flight DMA:
                #   front: evacuate yt PSUM + kick the denom-row fold DMAs
                #   mid (a pair later): reciprocal + kick the broadcast DMAs
                #   back (another pair later): normalize-multiply into yT
                fin_q1, fin_q2 = [], []

                def finish_front(h, c, yt, plo, phi):
                    ys = nrmpool.tile([D + 1, W], dt32, tag="ys")
                    nc.vector.tensor_copy(ys[:], yt[0:D + 1, :])
                    nc.sync.dma_start(
                        lsc_d.ap()[h, plo:phi].rearrange("(o t) -> o t", o=1),
                        ys[D:D + 1, :])
                    dn = nrmpool.tile([P, W // P], dt32, tag="dn")
                    nc.gpsimd.dma_start(
                        dn[:],
                        lsc_d.ap()[h, plo:phi].rearrange("(p c) -> p c", p=P))
                    fin_q1.append((h, ys, dn, plo, phi))

                def finish_mid(st):
                    h, ys, dn, plo, phi = st
                    nc.vector.reciprocal(dn[:], dn[:])
                    nc.gpsimd.dma_start(
                        lsc2_d.ap()[h, plo:phi].rearrange("(p c) -> p c", p=P),
                        dn[:])
                    bc = nrmpool.tile([D, W], dt32, tag="bc")
                    nc.sync.dma_start(
                        bc[:],
                        lsc2_d.ap()[h, plo:phi].rearrange(
                            "(o t) -> o t", o=1).broadcast_to([D, W]))
                    fin_q2.append((h, ys, bc, plo, phi))

                def finish_back(st):
                    h, ys, bc, plo, phi = st
                    r0 = (h % 2) * D
                    nc.vector.tensor_mul(
                        yT[r0:r0 + D, h // 2, plo:phi], ys[0:D, :], bc[:])

                def finish_step():
                    while len(fin_q1) > 2:
                        finish_mid(fin_q1.pop(0))
                    while len(fin_q2) > 2:
                        finish_back(fin_q2.pop(0))

                def finish_flush():
                    while fin_q1:
                        finish_mid(fin_q1.pop(0))
                    while fin_q2:
                        finish_back(fin_q2.pop(0))

                def finish_fast(h, yt, plo, phi):
                    """DMA-free normalize (gpsimd broadcast + fast DVE
                    reciprocal) — low latency, for the last column pass."""
                    drow = nrmpool.tile([1, W], dt32, tag="drow")
                    nc.vector.tensor_copy(drow[:], yt[D:D + 1, :])
                    ys = nrmpool.tile([D + 1, W], dt32, tag="ys")
                    nc.vector.tensor_copy(ys[0:D, :], yt[0:D, :])
                    bc = nrmpool.tile([D, W], dt32, tag="bc")
                    nc.gpsimd.partition_broadcast(bc[:], drow[:], channels=D)
                    rec = nrmpool.tile([D, W], dt32, tag="bc", name="rec")
                    nc.vector.reciprocal_approx_fast(out=rec[:], in_=bc[:])
                    r0 = (h % 2) * D
                    nc.vector.tensor_mul(
                        yT[r0:r0 + D, h // 2, plo:phi], ys[0:D, :], rec[:])

                def emit_proj_mms(tt, pos, k0, k1):
                    for kt in range(k0, k1):
                        for nn in range(2):
                            nc.tensor.matmul(
                                pos[nn][:],
                                yT[:, kt, tt * P:(tt + 1) * P],
                                wpsb[:, kt, nn * W:(nn + 1) * W],
                                start=(kt == 0), stop=(kt == NP - 1),
                                skip_group_check=True)

                def emit_proj_done(tt, pos):
                    ot = opool.tile([P, C], bf16, tag="ot")
                    for nn in range(2):
                        nc.vector.tensor_copy(
                            ot[:, nn * W:(nn + 1) * W], pos[nn][:])
                    [nc.sync, nc.gpsimd][tt % 2].dma_start(
                        out_d.ap()[tt * P:(tt + 1) * P, :], ot[:])

                def emit_proj_tt(tt):
                    # kernel-tail only: S traffic is done, reuse a psS slot
                    pop = psS.tile([P, 2, W], dt32, tag="s", name="pop")
                    pos = [pop[:, 0, :], pop[:, 1, :]]
                    emit_proj_mms(tt, pos, 0, NP)
                    emit_proj_done(tt, pos)

                # Global S-emission cursor kept ~14 chunks ahead of PV
                # consumption: the exp stream never starves at pair or pass
                # boundaries. Pass 0 S tiles were pre-built in stage B.
                sq = [(c2, m2, j2) for c2 in range(1, 4) for m2 in range(NP)
                      for j2 in range(4 * c2 + 4)
                      if not (c2 == 1 and m2 == 0 and j2 < 4)]
                sq_i = 0
                pts_q = {(0, m2): list(pass0_pts[m2]) for m2 in range(NP)}
                pts_q[1, 0] = list(pass1_pre)
                state = {"ahead": sum(len(v) for v in pts_q.values())}

                def pull_s():
                    c2, m2, j2 = sq[sq_i]
                    pt = emit_s(m2, j2, c2 * W, (c2 + 1) * W)
                    pts_q.setdefault((c2, m2), []).append((j2, pt))
                    state["ahead"] += 1

                # proj tiles of pass c-1 interleave with pass c, placed so
                # every yT write they read is already emitted
                proj_hooks = {}
                for c2 in range(1, 4):
                    proj_hooks[c2, 1] = [4 * (c2 - 1)]
                    proj_hooks[c2, 2] = [4 * (c2 - 1) + 1]
                    proj_hooks[c2, 3] = [4 * (c2 - 1) + 2, 4 * (c2 - 1) + 3]

                # all v pieces are ready to run from the start (they only
                # need xts/wv): enqueue them all up front so they trickle out
                # between S chunks instead of bunching at pass boundaries
                for tt in range(4, TT):
                    fillers.extend(
                        (tt, f) for f in make_v_pieces(tt))

                for c in range(4):
                    plo, phi = c * W, (c + 1) * W
                    jmax = 4 * c + 3
                    # v-tiles this pass reads must be emitted before its
                    # first PV; later fillers keep trickling
                    pump_v_upto(jmax)
                    for m in range(NP):
                        hA, hB = 2 * m, 2 * m + 1
                        ytA = psY.tile([MV, W], dt32, tag="yt", name="ytA")
                        ytB = psY.tile([MV, W], dt32, tag="yt", name="ytB")
                        while (len(pts_q.get((c, m), [])) < jmax + 1
                               and sq_i < len(sq)):
                            pull_s()
                            sq_i += 1
                            pump()
                        for pj, ppt in pts_q.pop((c, m)):
                            emit_pv(hA, pj, ppt, ytA, plo, phi, jmax)
                            emit_pv(hB, pj, ppt, ytB, plo, phi, jmax)
                            state["ahead"] -= 1
                            while state["ahead"] < 16 and sq_i < len(sq):
                                pull_s()
                                sq_i += 1
                                pump()
                        if c < 3 or m == 0:
                            finish_front(hA, c, ytA, plo, phi)
                            finish_front(hB, c, ytB, plo, phi)
                            finish_step()
                            for tt in proj_hooks.get((c, m), []):
                                fillers.extend(
                                    (-1, f) for f in make_proj_pieces(tt))
                        elif m < NP - 1:
                            # last pass: low-latency normalize, no DMA bounce
                            finish_fast(hA, ytA, plo, phi)
                            finish_fast(hB, ytB, plo, phi)
                            if m == 1:
                                finish_flush()
                            for tt in proj_hooks.get((c, m), []):
                                fillers.extend(
                                    (-1, f) for f in make_proj_pieces(tt))
                        else:
                            for tt in proj_hooks.get((c, m), []):
                                fillers.extend(
                                    (-1, f) for f in make_proj_pieces(tt))
                            pump(len(fillers))
                            # all other pass-3 yT writes are emitted: the
                            # final proj's first 3 kt keep the PE warm while
                            # the last normalize chain drains
                            pop12 = psS.tile([P, 2, W], dt32, tag="s",
                                             name="pop12")
                            pos12 = [pop12[:, 0, :], pop12[:, 1, :]]
                            emit_proj_mms(12, pos12, 0, NP - 1)
                            finish_fast(hA, ytA, plo, phi)
                            finish_fast(hB, ytB, plo, phi)
                            emit_proj_mms(12, pos12, NP - 1, NP)
                            emit_proj_done(12, pos12)
                for tt in range(13, TT):
                    emit_proj_tt(tt)
            outer.close()

    nc.compile()
    return nc


def make_core_inputs(x, W_attn, b_attn, W_proj, n_cores=8, HC=8, D=64):
    """Host-side sharding: per-core input dicts."""
    B, T, C = x.shape
    CO = HC * D
    NP = CO // P
    bf = _bf16_np()
    f8 = _f8_np()
    in_maps = []
    for c in range(n_cores):
        b = c // (n_cores // B)
        h0 = (c % (n_cores // B)) * HC
        lo = h0 * D
        bq = b_attn[lo:lo + CO]
        bv = b_attn[2 * C + lo:2 * C + lo + CO]
        xtb = np.ascontiguousarray(x[b].T)
        in_maps.append({
            "xt": xtb.astype(bf),
            "xt8": xtb.astype(f8),
            "wq": np.ascontiguousarray(
                W_attn[:, lo:lo + CO] * WS).astype(f8),
            "wk": np.ascontiguousarray(
                W_attn[:, C + lo:C + lo + CO] * WS).astype(f8),
            "wv": np.ascontiguousarray(
                W_attn[:, 2 * C + lo:2 * C + lo + CO]).astype(bf),
            "bq": np.ascontiguousarray(bq.reshape(NP, P).T),
            "bvb": np.tile(bv[None, :], (P, 1)),
            "wp": np.ascontiguousarray(W_proj[lo:lo + CO, :]).astype(bf),
        })
    return in_maps


_CACHE = {}


def _get_program():
    if "nc" not in _CACHE:
        _CACHE["nc"] = build_program()
    return _CACHE["nc"]


def run_on_cores(x, W_attn, b_attn, W_proj, b_proj, trace=False):
    """Returns (full output [B,T,C], BassKernelResults)."""
    from concourse.bass_utils import run_bass_kernel_spmd

    x = np.asarray(x, np.float32)
    W_attn = np.asarray(W_attn, np.float32)
    b_attn = np.asarray(b_attn, np.float32)
    W_proj = np.asarray(W_proj, np.float32)
    b_proj = np.asarray(b_proj, np.float32)

    nc = _get_program()
    in_maps = make_core_inputs(x, W_attn, b_attn, W_proj)
    res = run_bass_kernel_spmd(nc, in_maps, core_ids=list(range(8)), trace=trace)
    B, T, C = x.shape
    out = np.empty((B, T, C), np.float32)
    for b in range(B):
        out[b] = (res.results[2 * b]["out"].astype(np.float32)
                  + res.results[2 * b + 1]["out"].astype(np.float32)
                  + b_proj[None, :])
    return out, res


def kernel(x, W_attn, b_attn, W_proj, b_proj):
    out, _ = run_on_cores(x, W_attn, b_attn, W_proj, b_proj, trace=False)
    return out
